# revision 1
# baseline (speedup 1.0000x reference)
"""Trainium2 Bass kernel for nn_Conv1d_NN (KNN gather + conv1d).

Data-parallel over batch: 16 batches -> 8 cores, 2 batches/core.
Per batch (C=64, T=2048, K=3):
  1. scores S'[i,j] = 2*x_i.x_j - |x_j|^2  (argmax_j S' == argmin_j dist)
     computed as one fp32 PE matmul per 128-row chunk with an augmented
     contraction row: lhsT = [2x ; ones](65,128), rhs = [x ; -nsq](65,512).
  2. top-3 neighbours per row: DVE max8 + max_index (ties: lowest index
     first, matching jax.lax.top_k).
  3. indices -> DRAM bounce -> 16-partition "wrapped" layout (int16) for
     dma_gather; gather neighbour rows from xT (2048,64) in HBM.
  4. PE-transpose gathered (128,64) blocks back to channel-major, then
     conv = 3 accumulated (64x64)@(64x128) fp32 matmuls + bias.
"""

import os
import numpy as np

import concourse.bass as bass
import concourse.bacc as bacc
import concourse.mybir as mybir
from concourse.tile import TileContext
from concourse.masks import make_identity
from concourse import library_config
from concourse.bass_utils import run_bass_kernel_spmd

F32 = mybir.dt.float32
F32R = mybir.dt.float32r
U16 = mybir.dt.uint16
U32 = mybir.dt.uint32
I16 = mybir.dt.int16

B_PER_CORE = 2
C = 64
T = 2048
K = 3
NCHUNK = T // 128          # 16 row chunks
NIDX = T * K               # 6144 gathered tokens per batch


def build_program():
    nc = bacc.Bacc()

    x_d = nc.dram_tensor("x", [B_PER_CORE, C, T], F32, kind="ExternalInput")
    xT_ds = [nc.dram_tensor(f"xT{i}", [T, C], F32, kind="ExternalInput")
             for i in range(B_PER_CORE)]
    wt_d = nc.dram_tensor("wt", [K, C, C], F32, kind="ExternalInput")
    bias_d = nc.dram_tensor("bias", [C, 1], F32, kind="ExternalInput")
    out_d = nc.dram_tensor("out", [B_PER_CORE, C, T], F32, kind="ExternalOutput")

    with TileContext(nc) as tc:
        with (
            tc.tile_pool(name="const", bufs=1) as cpool,
            tc.tile_pool(name="aug", bufs=2) as augpool,
            tc.tile_pool(name="xsq", bufs=2) as xsqpool,
            tc.tile_pool(name="scores", bufs=3) as scpool,
            tc.tile_pool(name="vals", bufs=4) as vpool,
            tc.tile_pool(name="idx", bufs=2) as ipool,
            tc.tile_pool(name="gath", bufs=2) as gpool,
            tc.tile_pool(name="gr", bufs=3) as grpool,
            tc.tile_pool(name="osb", bufs=2) as opool_sb,
            tc.tile_pool(name="pnsq", bufs=1, space="PSUM") as npool,
            tc.tile_pool(name="psc", bufs=4, space="PSUM") as spool,
            tc.tile_pool(name="ptr", bufs=2, space="PSUM") as tpool,
            tc.tile_pool(name="pco", bufs=1, space="PSUM") as copool,
        ):
            ident = cpool.tile([128, 128], F32)
            make_identity(nc, ident)
            wt_sb = cpool.tile([C, K * C], F32)
            nc.sync.dma_start(
                out=wt_sb.rearrange("i (k o) -> i k o", k=K),
                in_=wt_d.rearrange("k i o -> i k o"),
            )
            bias_sb = cpool.tile([C, 1], F32)
            nc.sync.dma_start(out=bias_sb, in_=bias_d[:, :])
            negones = cpool.tile([C, 1], F32)
            nc.vector.memset(negones, -1.0)

            for b in range(B_PER_CORE):
                # ---- load x, build augmented operands ----
                augR = augpool.tile([C + 1, T], F32, tag="augR")
                nc.sync.dma_start(out=augR[0:C, :], in_=x_d[b])
                augL = augpool.tile([C + 1, T], F32, tag="augL")
                nc.scalar.mul(augL[0:C, :], augR[0:C, :], 2.0)
                nc.vector.memset(augL[C : C + 1, :], 1.0)

                xsq = xsqpool.tile([C, T], F32)
                nc.vector.tensor_mul(xsq, augR[0:C, :], augR[0:C, :])
                for q in range(4):
                    pn = npool.tile([1, 512], F32)
                    nc.tensor.matmul(
                        pn, negones, xsq[:, q * 512 : (q + 1) * 512],
                        start=True, stop=True,
                    )
                    nc.scalar.copy(augR[C : C + 1, q * 512 : (q + 1) * 512], pn)

                # ---- scores + top-8 scan per 128-row chunk ----
                idx_all = ipool.tile([128, NCHUNK, 8], U32)
                for r in range(NCHUNK):
                    s_r = scpool.tile([128, T], F32)
                    lhs = augL[:, r * 128 : (r + 1) * 128]
                    for q in range(4):
                        ps = spool.tile([128, 512], F32)
                        nc.tensor.matmul(
                            ps, lhs, augR[:, q * 512 : (q + 1) * 512],
                            start=True, stop=True,
                        )
                        nc.scalar.copy(s_r[:, q * 512 : (q + 1) * 512], ps)
                    vals = vpool.tile([128, 8], F32)
                    nc.vector.max(out=vals, in_=s_r)
                    nc.vector.max_index(
                        out=idx_all[:, r, :], in_max=vals, in_values=s_r
                    )

                # ---- gather neighbour features (token-major blocks) ----
                gath = gpool.tile([128, NIDX // 128, C], F32)
                for r in range(NCHUNK):
                    for k in range(K):
                        nc.gpsimd.indirect_dma_start(
                            out=gath[:, K * r + k, :],
                            out_offset=None,
                            in_=xT_ds[b][:, :],
                            in_offset=bass.IndirectOffsetOnAxis(
                                ap=idx_all[:, r, k : k + 1], axis=0
                            ),
                        )

                # ---- transpose back to channel-major + conv (fp32r, 512-wide) ----
                out_sb = opool_sb.tile([C, T], F32)
                for rr in range(NCHUNK // 4):
                    g4 = grpool.tile([C, 4 * K * 128], F32)
                    for rsub in range(4):
                        r = rr * 4 + rsub
                        pt = tpool.tile([C, K * 128], F32)
                        for k in range(K):
                            nc.tensor.transpose(
                                pt[:, k * 128 : (k + 1) * 128],
                                gath[:, K * r + k, :],
                                ident,
                            )
                        nc.scalar.copy(
                            g4[:, rsub * K * 128 : (rsub + 1) * K * 128], pt
                        )
                    po = copool.tile([C, 512], F32)
                    g4v = g4.rearrange("c (rs m) -> c rs m", rs=4)
                    for k in range(K):
                        nc.tensor.matmul(
                            po.rearrange("c (rs m) -> c rs m", rs=4),
                            wt_sb[:, k * C : (k + 1) * C],
                            g4v[:, :, k * 128 : (k + 1) * 128],
                            start=(k == 0),
                            stop=(k == K - 1),
                        )
                    nc.vector.tensor_scalar(
                        out=out_sb[:, rr * 512 : (rr + 1) * 512],
                        in0=po,
                        scalar1=bias_sb,
                        scalar2=None,
                        op0=mybir.AluOpType.add,
                    )
                nc.sync.dma_start(out=out_d[b], in_=out_sb)

    nc.finalize()
    return nc


_NC = None


def _get_nc():
    global _NC
    if _NC is None:
        _NC = build_program()
    return _NC


def make_in_maps(x, W, b):
    x = np.ascontiguousarray(np.asarray(x, dtype=np.float32))
    xT = np.ascontiguousarray(np.transpose(x, (0, 2, 1)))
    wt = np.ascontiguousarray(np.transpose(np.asarray(W, np.float32), (2, 1, 0)))
    bias = np.ascontiguousarray(np.asarray(b, np.float32).reshape(C, 1))
    in_maps = []
    for c in range(8):
        sl = slice(c * B_PER_CORE, (c + 1) * B_PER_CORE)
        m = {"x": x[sl], "wt": wt, "bias": bias}
        for i in range(B_PER_CORE):
            m[f"xT{i}"] = xT[c * B_PER_CORE + i]
        in_maps.append(m)
    return in_maps


def kernel(x, W, b):
    nc = _get_nc()
    in_maps = make_in_maps(x, W, b)
    res = run_bass_kernel_spmd(nc, in_maps, list(range(8))).results
    out = np.concatenate([res[c]["out"] for c in range(8)], axis=0)
    return np.ascontiguousarray(out.astype(np.float32))



# revision 8
# speedup vs baseline: 1.0366x; 1.0366x over previous
"""Trainium2 Bass kernel for nn_Conv1d_NN (KNN gather + conv1d).

Data-parallel over batch: 16 batches -> 8 cores, 2 batches/core.
Per batch (C=64, T=2048, K=3):
  1. scores S'[i,j] = 2*x_i.x_j - |x_j|^2 via fp32 PE matmuls with an
     augmented contraction row (lhsT=[2x;1], rhs=[x;-nsq], -nsq precomputed
     host-side), written to a 4-bank PSUM tile per 128-row chunk.
  2. top-8 per row: DVE max8 + max_index read PSUM directly (fp32 exact;
     ties resolve to lowest index, matching jax.lax.top_k).
  3. k=0 neighbour is always the token itself (dist=0), so only k=1,2 are
     gathered: indices bounce through a DRAM tile into the 16-partition
     wrapped int16 layout, then ONE dma_gather(transpose=True) per
     half-batch pulls neighbour rows from bf16 channel-padded xT and
     transposes them to channel-major bf16 on the fly.
  4. conv: per 512-col group, PSUM-accumulated matmuls
     k=0: fp32 [0.5*W0^T; bias] @ [2x; 1]  (bias folded via the ones row)
     k=1,2: bf16 W_k^T @ gathered; Act copy PSUM->SBUF; DMA out.
"""

import numpy as np
import ml_dtypes

import concourse.bass as bass
import concourse.bacc as bacc
import concourse.mybir as mybir
from concourse.tile import TileContext
from concourse.bass_utils import run_bass_kernel_spmd

F32 = mybir.dt.float32
BF16 = mybir.dt.bfloat16
U16 = mybir.dt.uint16
I16 = mybir.dt.int16

B_PER_CORE = 2
C = 64
T = 2048
K = 3
NCHUNK = T // 128          # 16 row chunks
CP = 128                   # channel-padded row length for the bf16 gather


def build_program():
    nc = bacc.Bacc()

    xaug_d = nc.dram_tensor("xaug", [B_PER_CORE, C + 1, T], F32,
                            kind="ExternalInput")
    xTp_ds = [nc.dram_tensor(f"xTp{i}", [T, CP], BF16, kind="ExternalInput")
              for i in range(B_PER_CORE)]
    wt0_d = nc.dram_tensor("wt0", [C + 1, C], F32, kind="ExternalInput")
    wt12_d = nc.dram_tensor("wt12", [C, 2 * C], BF16, kind="ExternalInput")
    out_d = nc.dram_tensor("out", [B_PER_CORE, C, T], F32, kind="ExternalOutput")

    with TileContext(nc) as tc:
        with (
            tc.tile_pool(name="const", bufs=1) as cpool,
            tc.tile_pool(name="augR", bufs=2) as apool,
            tc.tile_pool(name="augL", bufs=2) as lpool,
            tc.tile_pool(name="osb", bufs=2) as opool,
            tc.tile_pool(name="vals", bufs=4) as vpool,
            tc.tile_pool(name="idx", bufs=2) as ipool,
            tc.tile_pool(name="wrap", bufs=2) as wpool,
            tc.tile_pool(name="gath", bufs=4) as gpool,
            tc.tile_pool(name="didx", bufs=4, space="DRAM") as dpool,
            tc.tile_pool(name="ps", bufs=2, space="PSUM") as ppool,
        ):
            wt0_sb = cpool.tile([C + 1, C], F32)
            nc.sync.dma_start(out=wt0_sb, in_=wt0_d[:, :])
            wt12_sb = cpool.tile([C, 2 * C], BF16)
            nc.sync.dma_start(out=wt12_sb, in_=wt12_d[:, :])

            augLs, gaths = [], []
            for b in range(B_PER_CORE):
                # ---- load [x; -nsq], build lhsT [2x; 1] ----
                augR = apool.tile([C + 1, T], F32)
                nc.sync.dma_start(out=augR, in_=xaug_d[b])
                augL = lpool.tile([C + 1, T], F32)
                nc.scalar.mul(augL[0:C, :], augR[0:C, :], 2.0)
                nc.gpsimd.memset(augL[C : C + 1, :], 1.0)
                augLs.append(augL)

                # ---- scores + top-8 per 128-row chunk (PSUM-direct scans) ----
                idx_all = ipool.tile([128, NCHUNK, 8], U16)
                for r in range(NCHUNK):
                    psc = ppool.tile([128, T], F32, tag="sc")
                    lhs = augL[:, r * 128 : (r + 1) * 128]
                    for q in range(4):
                        nc.tensor.matmul(
                            psc[:, q * 512 : (q + 1) * 512],
                            lhs, augR[:, q * 512 : (q + 1) * 512],
                            start=True, stop=True,
                        )
                    vals = vpool.tile([128, 8], F32)
                    nc.vector.max(out=vals, in_=psc)
                    nc.vector.max_index(
                        out=idx_all[:, r, :], in_max=vals, in_values=psc
                    )

                # ---- index bounce to wrapped int16 + fused gather+transpose ----
                wr = wpool.tile([128, 2 * 128], I16)
                gb = []
                for h in range(2):
                    didx = dpool.tile([128, 8, 2], U16, tag="dx")
                    nc.sync.dma_start(
                        out=didx, in_=idx_all[:, h * 8 : (h + 1) * 8, 1:3]
                    )
                    src = didx.rearrange(
                        "(ph p2) ri k -> p2 (ri k) ph", ph=8
                    ).bitcast(I16)
                    for g in range(8):
                        eng = nc.sync if g % 2 == 0 else nc.scalar
                        dst = wr[
                            g * 16 : (g + 1) * 16, h * 128 : (h + 1) * 128
                        ].rearrange("p (rik ph) -> p rik ph", rik=16)
                        eng.dma_start(out=dst, in_=src)
                    gath = gpool.tile([CP, 1, 2048], BF16, tag=f"g{h}")
                    nc.gpsimd.dma_gather(
                        out_ap=gath,
                        in_ap=xTp_ds[b][:, :],
                        idxs_ap=wr[:, h * 128 : (h + 1) * 128],
                        num_idxs=2048,
                        num_idxs_reg=2048,
                        elem_size=CP,
                        transpose=True,
                    )
                    gb.append(gath)
                gaths.append(gb)

            # ---- conv: k=0 from augL (bias folded), k=1,2 from gathers ----
            for b in range(B_PER_CORE):
                out_sb = opool.tile([C, T], F32)
                for rr in range(4):
                    h, par = rr // 2, rr % 2
                    pot = ppool.tile([128, T], F32, tag="sc")
                    po = pot[0:C, 0:512]
                    cols = slice(rr * 512, (rr + 1) * 512)
                    nc.tensor.matmul(
                        po, wt0_sb, augLs[b][:, cols], start=True, stop=False
                    )
                    gv = gaths[b][h][0:C, 0, :].rearrange(
                        "c (ri kk p) -> c ri kk p", ri=8, kk=2
                    )
                    for kk in range(2):
                        rhs = gv[:, par * 4 : (par + 1) * 4, kk, :]
                        nc.tensor.matmul(
                            po, wt12_sb[:, kk * C : (kk + 1) * C], rhs,
                            start=False, stop=(kk == 1),
                        )
                    nc.scalar.copy(out_sb[:, cols], po)
                nc.sync.dma_start(out=out_d[b], in_=out_sb)

    nc.finalize()
    return nc


_NC = None


def _get_nc():
    global _NC
    if _NC is None:
        _NC = build_program()
    return _NC


def make_in_maps(x, W, b):
    x = np.ascontiguousarray(np.asarray(x, dtype=np.float32))
    W = np.asarray(W, np.float32)
    b = np.asarray(b, np.float32)
    B = x.shape[0]
    nsq = (x * x).sum(axis=1, dtype=np.float32)           # (B, T)
    xaug = np.concatenate([x, -nsq[:, None, :]], axis=1)  # (B, 65, T)
    xTp = np.zeros((B, T, CP), dtype=ml_dtypes.bfloat16)
    xTp[:, :, :C] = np.transpose(x, (0, 2, 1)).astype(ml_dtypes.bfloat16)
    wt0 = np.concatenate(
        [0.5 * W[:, :, 0].T, b.reshape(1, C)], axis=0
    ).astype(np.float32)
    wt12 = np.concatenate([W[:, :, 1].T, W[:, :, 2].T], axis=1).astype(
        ml_dtypes.bfloat16
    )
    in_maps = []
    for c in range(8):
        sl = slice(c * B_PER_CORE, (c + 1) * B_PER_CORE)
        m = {"xaug": xaug[sl], "wt0": wt0, "wt12": wt12}
        for i in range(B_PER_CORE):
            m[f"xTp{i}"] = xTp[c * B_PER_CORE + i]
        in_maps.append(m)
    return in_maps


def kernel(x, W, b):
    nc = _get_nc()
    in_maps = make_in_maps(x, W, b)
    res = run_bass_kernel_spmd(nc, in_maps, list(range(8))).results
    out = np.concatenate([res[c]["out"] for c in range(8)], axis=0)
    return np.ascontiguousarray(out.astype(np.float32))
